# revision 12
# baseline (speedup 1.0000x reference)
"""AdaptiveCoverageAttention TRN2 kernel: 8-way (batch x head-group) sharded.

Sharding: core c in 0..7 -> batch b = c//4, head-group hg = c%4 (4 heads each).
Each core computes its 4 heads' attention + its partial output projection;
the host sums the 4 partials per batch. No collectives needed.

All heavy matmuls run as float32r (TensorE full rate, ~11-bit mantissa,
operands pre-rounded on host with RNE dropping 12 mantissa LSBs).
Attention uses an S^T layout: S^T[j,i] tiles via K=64 matmuls, exp on ScalarE
(bias = g*cov[j,h] per-partition, scale = hd^-0.5), then O^T = [V|1]^T @ P^T
which yields the softmax denominator as row 64 for free.
"""
import sys

sys.path.insert(0, "/opt/trn_rl_repo")

import numpy as np

B, N, D, H = 2, 2048, 1024, 16
HD = D // H            # 64
HPC = 4                # heads per core
NCORES = 8

_COMPILED = {}


def _rne12(x):
    """Round f32 to float32r's grid: RNE dropping the low 12 mantissa bits.
    Verified bit-exact vs the DVE f32->f32r conversion on TRN2."""
    x = np.ascontiguousarray(x, dtype=np.float32)
    v = x.view(np.uint32).astype(np.uint64)
    d = np.uint64(12)
    half = np.uint64(1) << (d - np.uint64(1))
    lsb = (v >> d) & np.uint64(1)
    r = (v + half - np.uint64(1) + lsb) & (np.uint64(0xFFFFFFFFFFFFFFFF) << d)
    return (r & np.uint64(0xFFFFFFFF)).astype(np.uint32).view(np.float32)


def _bf16(x):
    import ml_dtypes
    return np.ascontiguousarray(np.asarray(x, np.float32)).astype(ml_dtypes.bfloat16)


def build(n=N):
    """Build the per-core Bass graph for sequence length n (n % 512 == 0)."""
    import contextlib

    import concourse.bacc as bacc
    import concourse.tile as tile
    from concourse import mybir

    f32 = mybir.dt.float32
    f32r = mybir.dt.float32r
    bf16 = mybir.dt.bfloat16
    AFT = mybir.ActivationFunctionType

    NI = n // 512          # 512-wide i-chunks
    NJ = n // 128          # 128-wide j-tiles
    DC = D // 128          # 8 contraction chunks

    nc = bacc.Bacc("TRN2", target_bir_lowering=False, debug=False,
                   num_devices=NCORES)

    dram = lambda name, shape, dt, kind: nc.dram_tensor(name, shape, dt, kind=kind).ap()
    XT = dram("xT", [D, n], bf16, "ExternalInput")
    WQK = dram("wqk", [D, 512], bf16, "ExternalInput")
    WV = dram("wv", [D, 256], bf16, "ExternalInput")
    WO = dram("wo", [256, D], bf16, "ExternalInput")
    COVT = dram("covT", [1, n], f32, "ExternalInput")
    WCE1 = dram("wce1", [1, 256], f32, "ExternalInput")
    BCE1 = dram("bce1", [128, 2], f32, "ExternalInput")
    WCE2 = dram("wce2", [128, 8], f32, "ExternalInput")
    BCE2 = dram("bce2", [128, 4], f32, "ExternalInput")
    WFG1 = dram("wfg1", [D, 256], f32, "ExternalInput")
    BFG1 = dram("bfg1", [128, 2], f32, "ExternalInput")
    WFG2 = dram("wfg2", [128, 2], f32, "ExternalInput")
    BFG2 = dram("bfg2", [1, 1], f32, "ExternalInput")
    BOUT = dram("bout", [128, D], f32, "ExternalInput")
    OUT = dram("out", [n, D], f32, "ExternalOutput")

    with tile.TileContext(nc) as tc, contextlib.ExitStack() as ctx:
        consts = ctx.enter_context(tc.tile_pool(name="consts", bufs=1))
        xtp = ctx.enter_context(tc.tile_pool(name="xtp", bufs=DC))
        qkv = ctx.enter_context(tc.tile_pool(name="qkv", bufs=1))
        big2 = ctx.enter_context(tc.tile_pool(name="big2", bufs=1))
        ep = ctx.enter_context(tc.tile_pool(name="ep", bufs=4))
        rp = ctx.enter_context(tc.tile_pool(name="rp", bufs=3))

        # ---- constants into SBUF ----
        wqk_sb = consts.tile([128, DC, 512], bf16)
        wv_sb = consts.tile([128, DC, 256], bf16)
        wo_sb = consts.tile([128, 2, D], bf16)
        covT_sb = consts.tile([1, n], f32)
        wce1_sb = consts.tile([1, 256], f32)
        bce1_sb = consts.tile([128, 2], f32)
        wce2_sb = consts.tile([128, 8], f32)
        bce2_sb = consts.tile([128, 4], f32)
        bfg1_sb = consts.tile([128, 2], f32)
        wfg2_sb = consts.tile([128, 2], f32)
        bfg2_sb = consts.tile([1, 1], f32)
        bout_sb = consts.tile([128, D], f32)
        for dc in range(DC):
            nc.sync.dma_start(out=wqk_sb[:, dc, :], in_=WQK[dc * 128:(dc + 1) * 128, :])
            nc.sync.dma_start(out=wv_sb[:, dc, :], in_=WV[dc * 128:(dc + 1) * 128, :])
        for pt in range(2):
            nc.sync.dma_start(out=wo_sb[:, pt, :], in_=WO[pt * 128:(pt + 1) * 128, :])
        nc.sync.dma_start(out=covT_sb, in_=COVT)
        nc.sync.dma_start(out=wce1_sb, in_=WCE1)
        nc.sync.dma_start(out=bce1_sb, in_=BCE1)
        nc.sync.dma_start(out=wce2_sb, in_=WCE2)
        nc.sync.dma_start(out=bce2_sb, in_=BCE2)
        nc.sync.dma_start(out=bfg1_sb, in_=BFG1)
        nc.sync.dma_start(out=wfg2_sb, in_=WFG2)
        nc.sync.dma_start(out=bfg2_sb, in_=BFG2)
        nc.sync.dma_start(out=bout_sb, in_=BOUT)

        ones_f = consts.tile([1, 128], f32)
        nc.vector.memset(ones_f, 1.0)
        onesb_f = consts.tile([128, 64], f32)
        nc.vector.memset(onesb_f, 1.0)
        onesb_r = consts.tile([128, 64], f32r)
        nc.vector.tensor_copy(onesb_r, onesb_f)
        onecol_f = consts.tile([128, 1], f32)
        nc.vector.memset(onecol_f, 1.0)

        pooled_sb = consts.tile([128, DC], f32)
        hidg_sb = consts.tile([128, 2], f32)
        g_sb = consts.tile([1, 1], f32)
        gb_sb = consts.tile([128, 1], f32)
        wce2g_sb = consts.tile([128, 8], f32)
        gbce2_sb = consts.tile([128, 4], f32)
        bias_sb = consts.tile([128, NJ, 4], f32)

        # ---- stage A: load xT, pooled sums, gate/cov MLPs, Q^T/K^T, V ----
        xts = []
        for dc in range(DC):
            xt = xtp.tile([128, n], bf16, tag="xt", name=f"xt{dc}")
            nc.sync.dma_start(out=xt, in_=XT[dc * 128:(dc + 1) * 128, :])
            xts.append(xt)

        with tc.tile_pool(name="psA", bufs=2, space="PSUM") as psA, \
             tc.tile_pool(name="pst", bufs=2, space="PSUM") as pst:
            for dc in range(DC):
                nc.vector.reduce_sum(pooled_sb[:, dc:dc + 1], xts[dc],
                                     axis=mybir.AxisListType.X)

            # gate MLP (tiny, plain f32 matmuls); wfg1 streamed per d-chunk
            pgs = [pst.tile([128, 512], f32, tag="tiny", name=f"pg{i}")
                   for i in range(2)]
            for dc in range(DC):
                wf = rp.tile([128, 256], f32, tag="wfg1", name=f"wf{dc}")
                nc.sync.dma_start(out=wf, in_=WFG1[dc * 128:(dc + 1) * 128, :])
                for mc in range(2):
                    nc.tensor.matmul(pgs[mc][:, 0:1], wf[:, mc * 128:(mc + 1) * 128],
                                     pooled_sb[:, dc:dc + 1],
                                     start=(dc == 0), stop=(dc == DC - 1))
            for mc in range(2):
                nc.scalar.activation(out=hidg_sb[:, mc:mc + 1], in_=pgs[mc][:, 0:1],
                                     func=AFT.Silu, bias=bfg1_sb[:, mc:mc + 1],
                                     scale=1.0 / n)
            pgp = pst.tile([128, 512], f32, tag="tiny")
            for mc in range(2):
                nc.tensor.matmul(pgp[0:1, 0:1], hidg_sb[:, mc:mc + 1],
                                 wfg2_sb[:, mc:mc + 1],
                                 start=(mc == 0), stop=(mc == 1))
            nc.scalar.activation(out=g_sb, in_=pgp[0:1, 0:1], func=AFT.Sigmoid,
                                 bias=bfg2_sb, scale=1.0)
            pgb = pst.tile([128, 512], f32, tag="tiny")
            nc.tensor.matmul(pgb[:, 0:1], ones_f, g_sb, start=True, stop=True)
            nc.vector.tensor_copy(gb_sb, pgb[:, 0:1])

            # coverage MLP (tiny, plain f32): hidden^T then cov (scaled by g)
            hidc_sb = big2.tile([128, 2, n], f32, tag="big", name="hidc")
            for mc in range(2):
                for jc in range(NI):
                    ph = pst.tile([128, 512], f32, tag="tiny")
                    nc.tensor.matmul(ph, wce1_sb[:, mc * 128:(mc + 1) * 128],
                                     covT_sb[:, jc * 512:(jc + 1) * 512],
                                     start=True, stop=True)
                    nc.scalar.activation(out=hidc_sb[:, mc, jc * 512:(jc + 1) * 512],
                                         in_=ph, func=AFT.Silu,
                                         bias=bce1_sb[:, mc:mc + 1], scale=1.0)
            nc.vector.tensor_scalar_mul(out=wce2g_sb, in0=wce2_sb, scalar1=gb_sb)
            nc.vector.tensor_scalar_mul(out=gbce2_sb, in0=bce2_sb, scalar1=gb_sb)
            for jt in range(NJ):
                pc = pst.tile([128, 512], f32, tag="tiny")
                for mc in range(2):
                    nc.tensor.matmul(pc[:, 0:4], hidc_sb[:, mc, jt * 128:(jt + 1) * 128],
                                     wce2g_sb[:, mc * 4:(mc + 1) * 4],
                                     start=(mc == 0), stop=(mc == 1))
                nc.vector.tensor_add(bias_sb[:, jt, :], pc[:, 0:4], gbce2_sb)

            # Q^T / K^T: [col, i] = sum_d wqk[d, col] * xT[d, i]
            qt_sb = qkv.tile([128, 2, n], bf16)
            kt_sb = qkv.tile([128, 2, n], bf16)
            for cb in (0, 2, 1, 3):
                for ic in range(NI):
                    pq = psA.tile([128, 512], f32, tag="qk")
                    for dc in range(DC):
                        nc.tensor.matmul(pq, wqk_sb[:, dc, cb * 128:(cb + 1) * 128],
                                         xts[dc][:, ic * 512:(ic + 1) * 512],
                                         start=(dc == 0), stop=(dc == DC - 1))
                    dst = qt_sb if cb < 2 else kt_sb
                    nc.any.tensor_copy(dst[:, cb % 2, ic * 512:(ic + 1) * 512], pq)

            # V (natural layout) + ones column -> Vaug [j, 4*(64+1)]
            vaug_sb = qkv.tile([128, NJ, 4 * 65], bf16)
            for it in range(NJ):
                pv = psA.tile([128, 256], f32, tag="v")
                for dc in range(DC):
                    nc.tensor.matmul(pv, xts[dc][:, it * 128:(it + 1) * 128],
                                     wv_sb[:, dc, :],
                                     start=(dc == 0), stop=(dc == DC - 1))
                for h in range(HPC):
                    nc.any.tensor_copy(vaug_sb[:, it, h * 65:h * 65 + 64],
                                       pv[:, h * 64:(h + 1) * 64])
                    nc.any.tensor_copy(vaug_sb[:, it, h * 65 + 64:h * 65 + 65],
                                       onecol_f)

        # ---- attention: head pairs, S^T -> exp -> [1|V]^T P^T ----
        IC_W = 1024 if n >= 1024 else n
        NI2 = n // IC_W
        NS = IC_W // 512
        attn_sb = big2.tile([128, 2, n], bf16, tag="big", name="attn")
        with tc.tile_pool(name="pss", bufs=2, space="PSUM") as pss, \
             tc.tile_pool(name="pso", bufs=2, space="PSUM") as pso, \
             tc.tile_pool(name="unp", bufs=2 * NI2 + 2) as unp:
            for p in range(2):
                unns = {}
                dstacks = []
                for ic in range(NI2):
                    ds = rp.tile([128, IC_W], f32, tag="dstack",
                                 name=f"dstack{p}_{ic}")
                    nc.vector.memset(ds, 1.0)
                    dstacks.append(ds)
                for ic in range(NI2):
                    po = [pso.tile([128, IC_W], f32, tag="o", name=f"po{p}_{ic}_{i}")
                          for i in range(2)]
                    for jt in range(NJ):
                        for hh in range(2):
                            h = 2 * p + hh
                            lo = hh * 64
                            ps_ = pss.tile([128, IC_W], f32, tag="s")
                            for q in range(NS):
                                nc.tensor.matmul(
                                    ps_[:, q * 512:(q + 1) * 512],
                                    kt_sb[lo:lo + 64, p, jt * 128:(jt + 1) * 128],
                                    qt_sb[lo:lo + 64, p,
                                          ic * IC_W + q * 512:ic * IC_W + (q + 1) * 512],
                                    start=True, stop=True)
                            e = ep.tile([128, IC_W], bf16, tag="e")
                            nc.scalar.activation(out=e, in_=ps_, func=AFT.Exp,
                                                 bias=bias_sb[:, jt, h:h + 1],
                                                 scale=float(HD) ** -0.5)
                            for q in range(NS):
                                nc.tensor.matmul(
                                    po[hh][0:65, q * 512:(q + 1) * 512],
                                    vaug_sb[:, jt, h * 65:(h + 1) * 65],
                                    e[:, q * 512:(q + 1) * 512],
                                    start=(jt == 0), stop=(jt == NJ - 1))
                    for hh in range(2):
                        r = ic * 2 + hh
                        unn = unp.tile([65, IC_W], f32, tag="unn",
                                       name=f"unn{p}_{ic}_{hh}")
                        nc.vector.tensor_copy(unn, po[hh][0:65, :])
                        nc.sync.dma_start(out=dstacks[ic][32 * hh:32 * hh + 1, :],
                                          in_=unn[64:65, :])
                        unns[r] = unn
                # batched reciprocal of the denominator rows (partitions 0/64)
                drecrs = []
                for ic in range(NI2):
                    drec = rp.tile([128, IC_W], f32, tag="drec", name=f"drec{p}_{ic}")
                    nc.vector.reciprocal(drec, dstacks[ic])
                    drecr = rp.tile([128, IC_W], f32r, tag="drecr",
                                    name=f"drecr{p}_{ic}")
                    nc.vector.tensor_copy(drecr, drec)
                    drecrs.append(drecr)
                for ic in range(NI2):
                    for hh in range(2):
                        r = ic * 2 + hh
                        lo = hh * 64
                        pr = pss.tile([128, IC_W], f32, tag="s", name=f"pr{p}_{r}")
                        for q in range(NS):
                            nc.tensor.matmul(
                                pr[0:64, q * 512:(q + 1) * 512],
                                onesb_r[32 * hh:32 * hh + 1, :],
                                drecrs[ic][32 * hh:32 * hh + 1,
                                           q * 512:(q + 1) * 512],
                                start=True, stop=True)
                        nc.vector.tensor_mul(
                            attn_sb[lo:lo + 64, p, ic * IC_W:(ic + 1) * IC_W],
                            unns[r][0:64, :], pr[0:64, :])

        # ---- output projection: y[i, e] = sum_dim attnT[dim, i] wo[dim, e] ----
        with tc.tile_pool(name="psy", bufs=2, space="PSUM") as psy:
            for it in range(NJ):
                py = psy.tile([128, D], f32, tag="y")
                for half in range(2):
                    for pt in range(2):
                        nc.tensor.matmul(
                            py[:, half * 512:(half + 1) * 512],
                            attn_sb[:, pt, it * 128:(it + 1) * 128],
                            wo_sb[:, pt, half * 512:(half + 1) * 512],
                            start=(pt == 0), stop=(pt == 1))
                y_sb = xtp.tile([128, D], f32, tag="xt", name=f"ysb{it}")
                nc.vector.tensor_add(y_sb, py, bout_sb)
                nc.sync.dma_start(out=OUT[it * 128:(it + 1) * 128, :], in_=y_sb)

    nc.compile()
    return nc


def make_in_maps(x, coverage, w_qkv, w_out, b_out, w_ce1, b_ce1, w_ce2, b_ce2,
                 w_fg1, b_fg1, w_fg2, b_fg2, n=N):
    f = np.float32
    x = np.asarray(x, f)
    coverage = np.asarray(coverage, f)
    w_qkv = np.asarray(w_qkv, f)
    w_out = np.asarray(w_out, f)
    in_maps = []
    for c in range(NCORES):
        b, hg = divmod(c, 4)
        cs, ce = hg * 256, (hg + 1) * 256
        wq = w_qkv[:, 0 * D + cs:0 * D + ce]
        wk = w_qkv[:, 1 * D + cs:1 * D + ce]
        wv = w_qkv[:, 2 * D + cs:2 * D + ce]
        m = {
            "xT": _bf16(x[b].T),
            "wqk": _bf16(np.concatenate([wq, wk], axis=1)),
            "wv": _bf16(wv),
            "wo": _bf16(w_out[cs:ce, :]),
            "covT": np.ascontiguousarray(coverage[b, :, 0][None, :], f),
            "wce1": np.ascontiguousarray(np.asarray(w_ce1, f)),
            "bce1": np.ascontiguousarray(np.asarray(b_ce1, f).reshape(2, 128).T),
            "wce2": np.ascontiguousarray(
                np.asarray(w_ce2, f)[:, 4 * hg:4 * hg + 4].reshape(2, 128, 4)
                .transpose(1, 0, 2).reshape(128, 8)),
            "bce2": np.tile(np.asarray(b_ce2, f)[4 * hg:4 * hg + 4][None, :], (128, 1)),
            "wfg1": np.ascontiguousarray(np.asarray(w_fg1, f)),
            "bfg1": np.ascontiguousarray(np.asarray(b_fg1, f).reshape(2, 128).T),
            "wfg2": np.ascontiguousarray(np.asarray(w_fg2, f).reshape(2, 128).T),
            "bfg2": np.asarray(b_fg2, f).reshape(1, 1),
            "bout": (np.tile(np.asarray(b_out, f)[None, :], (128, 1))
                     if hg == 0 else np.zeros((128, D), f)),
        }
        in_maps.append(m)
    return in_maps


def kernel(**inputs):
    from concourse.bass_utils import run_bass_kernel_spmd
    if "nc" not in _COMPILED:
        _COMPILED["nc"] = build(N)
    nc = _COMPILED["nc"]
    in_maps = make_in_maps(**inputs)
    res = run_bass_kernel_spmd(nc, in_maps, core_ids=list(range(NCORES)))
    outs = [res.results[c]["out"] for c in range(NCORES)]
    full = np.stack([
        outs[0] + outs[1] + outs[2] + outs[3],
        outs[4] + outs[5] + outs[6] + outs[7],
    ]).astype(np.float32)
    return full
